# revision 25
# baseline (speedup 1.0000x reference)
"""Trainium2 Bass kernel for nn_MultiHeadAttention_77360950936277.

Reference computation (B=8, T=2048, C=64, H=4 heads, dh=64):
    Q = x@W1; K = x@W2; V = x@W3
    scores_h = Q_h K_h^T / 64      -> softmax over keys -> ctx_h = att_h V_h
    gate = concat_h(ctx_h) @ Wout ; out = x * gate

Kernel strategy (data-parallel: 1 batch element per NeuronCore, 8 cores):
  * Weight folding (host, exact algebra):
      A2_h = W2_h @ W1_h^T  -> scoresT_h = (x A2_h) x^T / 64   [k, q] layout
      u_h  = x @ (W3_h Wout_h)
      gate = sum_h (E_h^T u_h) / (E_h^T 1),  E_h = exp(scoresT_h - c_h)
    (c_h is a per-head shift; it cancels in the ratio and keeps E in a
     narrow range ~[64, 128].)
  * Host precomputes x^T (f16, stacked twice) and u (f16) so the device
    does no input transposes; all DMAs overlap the main stream.
  * Main loop over (key-tile, head-pair) units x 2 q-halves, software
    pipelined: scoresT chunks on PE (fp16, f32 psum), exp on ACT, E (f16)
    streamed back through PE against stationary [u_h | 1] accumulating
    g/rs rows in psum.  Half the odd-kt exp tiles are offloaded from ACT
    to DVE using E ~= (a*s + b)^2 (valid because the score range is tiny,
    |s|/64 < 0.45; the quadratic's rel err ~7e-3 on those tiles only).
  * zt prep and the first q-half's tail (PE transposes of g/rs, gate
    division, out = x * gate, output DMA) are interleaved into the
    stream; psum is partitioned 8KB scores + 4KB g/rs + 4KB shared by
    zt-prep and tail transposes so nothing aliases the hot pools.
"""

import numpy as np

from concourse import bacc, tile
import concourse.mybir as mybir
from concourse.alu_op_type import AluOpType
from concourse.bass_utils import run_bass_kernel_spmd

T = 2048
C = 64
H = 4
F = 256
P = 128
NT = T // P  # 16 key tiles

f32 = mybir.dt.float32
f16 = mybir.dt.float16
AF = mybir.ActivationFunctionType

_NC_CACHE = None


def _offloaded(kt, head):
    # 16 of 64 exp tiles per q-half move from ACT to DVE
    return kt % 2 == 1 and head % 2 == 1


def _build_nc():
    nc = bacc.Bacc("TRN2", target_bir_lowering=False, debug=False)
    x_d = nc.dram_tensor("x", [T, C], f32, kind="ExternalInput").ap()
    xt_d = nc.dram_tensor("xt2", [P, T], f16, kind="ExternalInput").ap()
    zt_d = [
        nc.dram_tensor(f"zt{i}", [P, T], f16, kind="ExternalInput").ap()
        for i in range(2)
    ]
    u_d = nc.dram_tensor("u16", [P, NT, 2, H], f16, kind="ExternalInput").ap()
    id_d = nc.dram_tensor("ident", [P, P], f32, kind="ExternalInput").ap()
    cf_d = nc.dram_tensor("coef", [P, 12], f32, kind="ExternalInput").ap()
    y_d = nc.dram_tensor("y", [T, C], f32, kind="ExternalOutput").ap()

    with tile.TileContext(nc) as tc:
        with tc.tile_pool(name="per", bufs=1) as per:
            xT2 = per.tile([P, T], f16, tag="xT2")
            x_sb = per.tile([P, NT, C], f32, tag="x_sb")
            u16 = per.tile([P, NT, 2, H], f16, tag="u16")
            id_sb = per.tile([P, P], f32, tag="id_sb")
            cf = per.tile([P, 12], f32, tag="cf")
            zt = [
                per.tile([P, T], f16, tag=f"zt{i}", name=f"zt{i}") for i in range(2)
            ]
            z1 = per.tile([1, P], f16, tag="z1")
            z512 = per.tile([1, 512], f16, tag="z512")
            t_sb = per.tile([P, 2, 1024], f32, tag="t_sb")
            gate = per.tile([P, NT], f32, tag="gate")
            y_sb = per.tile([P, NT, C], f32, tag="y_sb")
            warm = per.tile([P, 1], f32, tag="warm")

            # DMA order: the stream's critical path spread over 4 queues
            # (xT chunks 0/1 and a2h feed the first unit); x (tail-only)
            # last.
            nc.sync.dma_start(zt[0][:, 0:128], zt_d[0][:, 0:128])
            nc.gpsimd.dma_start(zt[1][:, 0:128], zt_d[1][:, 0:128])
            nc.scalar.dma_start(xT2[:, 0:512], xt_d[:, 0:512])
            nc.sync.dma_start(cf[:], cf_d[:])
            nc.sync.dma_start(zt[0][:, 128:512], zt_d[0][:, 128:512])
            nc.gpsimd.dma_start(zt[1][:, 128:512], zt_d[1][:, 128:512])
            nc.scalar.dma_start(xT2[:, 512:1024], xt_d[:, 512:1024])
            nc.sync.dma_start(u16[:], u_d[:])
            # bulk, ordered by first-use deadline (kt4 ~18us, kt8 ~26us,
            # q-pass 1 ~70us, tail ~75us)
            nc.sync.dma_start(zt[0][:, 512:1024], zt_d[0][:, 512:1024])
            nc.gpsimd.dma_start(zt[1][:, 512:1024], zt_d[1][:, 512:1024])
            nc.sync.dma_start(zt[0][:, 1024:2048], zt_d[0][:, 1024:2048])
            nc.gpsimd.dma_start(zt[1][:, 1024:2048], zt_d[1][:, 1024:2048])
            nc.scalar.dma_start(xT2[:, 1024:2048], xt_d[:, 1024:2048])
            nc.gpsimd.dma_start(id_sb[:], id_d[:])
            for g in range(4):
                q = [nc.sync, nc.gpsimd, nc.scalar, nc.scalar][g]
                q.dma_start(
                    x_sb[:, g * 4:(g + 1) * 4, :],
                    x_d[g * 512:(g + 1) * 512, :].rearrange("(j p) c -> p j c", p=P),
                )

            # Load the exp table on ACT early (~2.7us) so it overlaps prep.
            nc.vector.memset(warm[:], 0.0)
            nc.scalar.activation(warm[:], warm[:], AF.Exp, scale=1.0)

            nc.vector.memset(z1[:], 0.0)
            nc.vector.memset(z512[:], 0.0)


            with (
                tc.tile_pool(name="ps_s", bufs=3, space="PSUM") as pss,
                tc.tile_pool(name="ps_grs", bufs=1, space="PSUM") as psg,
                tc.tile_pool(name="e_pool", bufs=8) as ep,
                tc.tile_pool(name="t_pool", bufs=4) as tp,
                tc.tile_pool(name="tailsb", bufs=2) as tsb,
            ):
                pending_i2 = []

                def flush_i2():
                    while pending_i2:
                        e, t16 = pending_i2.pop(0)
                        nc.vector.tensor_mul(e[:], t16[:], t16[:])

                def emit_exp(ps, kt, h):
                    e = ep.tile([P, 1024], f16, tag="e", name="e")
                    if _offloaded(kt, h):
                        # i1 frees the psum slot; the square (i2) is
                        # deferred so i1s keep DVE-queue priority.
                        t16 = tp.tile([P, 1024], f16, tag="t16", name="t16")
                        nc.vector.tensor_scalar(
                            t16[:],
                            ps[:],
                            cf[:, 4 + h:5 + h],
                            cf[:, 8 + h:9 + h],
                            AluOpType.mult,
                            AluOpType.add,
                        )
                        pending_i2.append((e, t16))
                    else:
                        nc.scalar.activation(
                            e[:], ps[:], AF.Exp, bias=cf[:, h:h + 1], scale=1.0 / 64.0
                        )
                    return e

                def emit_scores_exp(qpass, kt, pair, split_act=False):
                    psA = pss.tile([P, 1024], f32, tag="ps_s", name="psA")
                    psB = pss.tile([P, 1024], f32, tag="ps_s", name="psB")
                    if split_act:
                        eA = ep.tile([P, 1024], f16, tag="e", name="e")
                        eB = ep.tile([P, 1024], f16, tag="e", name="e")
                    for sub in range(2):
                        q0 = qpass * 1024 + sub * 512
                        sl = slice(sub * 512, (sub + 1) * 512)
                        nc.tensor.matmul(
                            psA[:, sl],
                            zt[pair][0:C, kt * P:(kt + 1) * P],
                            xT2[0:C, q0:q0 + 512],
                            start=True,
                            stop=True,
                        )
                        nc.tensor.matmul(
                            psB[:, sl],
                            zt[pair][C:P, kt * P:(kt + 1) * P],
                            xT2[C:P, q0:q0 + 512],
                            start=True,
                            stop=True,
                        )
                        if split_act:
                            for e, ps, h in ((eA, psA, 2 * pair),
                                             (eB, psB, 2 * pair + 1)):
                                nc.scalar.activation(
                                    e[:, sl], ps[:, sl], AF.Exp,
                                    bias=cf[:, h:h + 1], scale=1.0 / 64.0,
                                )
                    if not split_act:
                        eA = emit_exp(psA, kt, 2 * pair)
                        eB = emit_exp(psB, kt, 2 * pair + 1)
                    return eA, eB

                def emit_pass2(grs, kt, pair, eA, eB, last, first=False):
                    flush_i2()
                    hA, hB = 2 * pair, 2 * pair + 1
                    for sub in range(2):
                        nc.tensor.matmul(
                            grs[32 * hA:32 * hA + 2, sub * 512:(sub + 1) * 512],
                            u16[:, kt, :, hA],
                            eA[:, sub * 512:(sub + 1) * 512],
                            start=False,
                            stop=last,
                            skip_group_check=True,
                            tile_position=(0, 32 * hA),
                        )
                        nc.tensor.matmul(
                            grs[32 * hB:32 * hB + 2, sub * 512:(sub + 1) * 512],
                            u16[:, kt, :, hB],
                            eB[:, sub * 512:(sub + 1) * 512],
                            start=False,
                            stop=last,
                            skip_group_check=True,
                            tile_position=(0, 32 * hB),
                        )

                def emit_tail(qt0, n, queues, per_tile_dma):
                    half = qt0 // 8
                    tg = pss.tile([P, n, P], f32, tag="ps_s", name="tg")
                    for j in range(n):
                        lcl = qt0 + j - half * 8
                        nc.tensor.transpose(
                            tg[:, j, :],
                            t_sb[:, half, lcl * P:(lcl + 1) * P],
                            id_sb[:],
                        )
                    tgr = tg[:].rearrange("p f (h j) -> p f h j", h=4)
                    rec = tsb.tile([P, n, H], f32, tag="rec", name="rec", bufs=2)
                    nc.vector.reciprocal(rec[:], tgr[:, :, :, 1])
                    gm = tsb.tile([P, n, H], f32, tag="gm", name="gm", bufs=2)
                    nc.vector.tensor_mul(gm[:], tgr[:, :, :, 0], rec[:])
                    nc.vector.tensor_reduce(
                        gate[:, qt0:qt0 + n],
                        gm[:],
                        axis=mybir.AxisListType.X,
                        op=mybir.AluOpType.add,
                    )
                    for j in range(n):
                        qt = qt0 + j
                        nc.vector.tensor_scalar_mul(
                            y_sb[:, qt, :], x_sb[:, qt, :], gate[:, qt:qt + 1]
                        )
                        if per_tile_dma:
                            queues[j % len(queues)].dma_start(
                                y_d[qt * P:(qt + 1) * P, :], y_sb[:, qt, :]
                            )
                    if not per_tile_dma:
                        queues[0].dma_start(
                            y_d[qt0 * P:(qt0 + n) * P, :].rearrange(
                                "(j p) c -> p j c", p=P
                            ),
                            y_sb[:, qt0:qt0 + n, :],
                        )

                for qpass in range(2):
                    grs = psg.tile([P, 1024], f32, tag="grs", name="grs")

                    def emit_seeds():
                        for c in range(2):
                            nc.tensor.matmul(
                                grs[:, c * 512:(c + 1) * 512],
                                z1[:],
                                z512[:],
                                start=True,
                                stop=False,
                                skip_group_check=True,
                            )

                    if qpass == 0:
                        emit_seeds()
                    units = [(kt, pair) for kt in range(NT) for pair in range(2)]
                    prev = None
                    for idx, unit in enumerate(units):
                        e_tiles = emit_scores_exp(
                            qpass, *unit, split_act=(qpass == 0 and idx < 2)
                        )
                        if qpass == 1 and idx == 0:
                            emit_seeds()  # deferred past the evac WAR
                        if qpass == 1 and idx == 4:
                            # half-0 tail overlaps q-pass 1
                            emit_tail(0, 4, [nc.sync], False)
                        if qpass == 1 and idx == 8:
                            emit_tail(4, 4, [nc.gpsimd], False)
                        if prev is not None:
                            emit_pass2(grs, *prev[0], *prev[1], last=False)
                        prev = (unit, e_tiles)
                    emit_pass2(grs, *prev[0], *prev[1], last=True)
                    if qpass == 0:
                        nc.vector.tensor_copy(t_sb[:, 0, :], grs[:])
                    else:
                        # fine-grained evac + 2-tile tail groups: shortest
                        # possible serial chain after the last exp
                        for gg in range(4):
                            c0 = gg * 256
                            nc.vector.tensor_copy(
                                t_sb[:, 1, c0:c0 + 256], grs[:, c0:c0 + 256]
                            )
                            emit_tail(8 + 2 * gg, 2,
                                      [nc.sync, nc.scalar], True)

    nc.compile()
    return nc


def _get_nc():
    global _NC_CACHE
    if _NC_CACHE is None:
        _NC_CACHE = _build_nc()
    return _NC_CACHE


def _host_prep(inputs_tran, W1, W2, W3, Wout):
    x64 = inputs_tran.astype(np.float64)
    W1r = W1.astype(np.float64).reshape(C, H, C)
    W2r = W2.astype(np.float64).reshape(C, H, C)
    W3r = W3.astype(np.float64).reshape(C, H, C)
    Wor = Wout.astype(np.float64).reshape(H, C)
    a2 = np.einsum("chd,qhd->chq", W2r, W1r)  # [C, H, Cq]
    wt = np.einsum("chd,hd->ch", W3r, Wor)    # [C, H]

    # Per-head score range (s/64) estimated on a q-subsample; the margins
    # below cover the sampling shortfall many times over (scores are
    # Gaussian-ish with sigma ~0.07 in s/64 units).
    z = np.einsum("btc,chq->bthq", x64, a2)   # [B, T, H, C]
    qsel = np.arange(0, T, 8)
    xs = x64[:, qsel, :]                      # [B, 256, C]
    smax = np.zeros(H)
    smin = np.zeros(H)
    for h in range(H):
        ss = np.einsum("btq,bsq->bts", z[:, :, h, :], xs) / 64.0
        smax[h] = ss.max()
        smin[h] = ss.min()

    ln_peak = np.log(128.0)
    coef = np.zeros((P, 12), dtype=np.float32)
    for h in range(H):
        c_h = smax[h] + 0.05 - ln_peak
        lo = smin[h] - c_h - 0.10
        hi = ln_peak + 0.10
        # fit exp(t) ~= (a t + b)^2 on [lo, hi]: weighted lstsq of a*t+b
        # against exp(t/2) (near-minimax in relative error)
        ts = np.linspace(lo, hi, 2001)
        y = np.exp(ts / 2.0)
        A = np.stack([ts / y, 1.0 / y], axis=1)
        (a_h, b_h), *_ = np.linalg.lstsq(A, np.ones_like(ts), rcond=None)
        coef[:, h] = -c_h                      # ACT exp bias
        coef[:, 4 + h] = a_h / 64.0            # DVE quad scale (raw psum s)
        coef[:, 8 + h] = b_h - a_h * c_h       # DVE quad offset

    # zt[b][fh] = [z_{2fh}^T ; z_{2fh+1}^T]  [128, T] f16 per batch
    ztp = np.empty((x64.shape[0], 2, P, T), dtype=np.float16)
    for fh in range(2):
        ztp[:, fh, 0:C, :] = z[:, :, 2 * fh, :].transpose(0, 2, 1)
        ztp[:, fh, C:P, :] = z[:, :, 2 * fh + 1, :].transpose(0, 2, 1)
    u = np.einsum("btc,ch->bth", x64, wt)      # [B, T, H]
    return ztp, u, coef


def _run(inputs_tran, W1, W2, W3, Wout, trace=False):
    nc = _get_nc()
    ztp, u, coef = _host_prep(inputs_tran, W1, W2, W3, Wout)
    ident = np.eye(P, dtype=np.float32)
    B = inputs_tran.shape[0]
    in_maps = []
    for b in range(B):
        xb = np.ascontiguousarray(inputs_tran[b], dtype=np.float32)
        xt2 = np.concatenate([xb.T, xb.T], axis=0).astype(np.float16)  # [128, T]
        u16 = np.empty((P, NT, 2, H), dtype=np.float16)
        # u16[p, kt, 0, h] = u_h at key kt*128+p
        u16[:, :, 0, :] = u[b].reshape(NT, P, H).transpose(1, 0, 2).astype(np.float16)
        u16[:, :, 1, :] = np.float16(1.0)
        in_maps.append(
            {
                "x": xb,
                "xt2": xt2,
                "zt0": np.ascontiguousarray(ztp[b, 0]),
                "zt1": np.ascontiguousarray(ztp[b, 1]),
                "u16": u16,
                "ident": ident,
                "coef": coef,
            }
        )
    res = run_bass_kernel_spmd(nc, in_maps, list(range(B)), trace=trace)
    out = np.stack([res.results[b]["y"] for b in range(B)], axis=0)
    return out.astype(np.float32), res


def kernel(inputs_tran, W1, W2, W3, Wout):
    out, _ = _run(inputs_tran, W1, W2, W3, Wout, trace=False)
    return out


# revision 27
# speedup vs baseline: 1.0335x; 1.0335x over previous
"""Trainium2 Bass kernel for nn_MultiHeadAttention_77360950936277.

Reference computation (B=8, T=2048, C=64, H=4 heads, dh=64):
    Q = x@W1; K = x@W2; V = x@W3
    scores_h = Q_h K_h^T / 64      -> softmax over keys -> ctx_h = att_h V_h
    gate = concat_h(ctx_h) @ Wout ; out = x * gate

Kernel strategy (data-parallel: 1 batch element per NeuronCore, 8 cores):
  * Weight folding (host, exact algebra):
      A2_h = W2_h @ W1_h^T  -> scoresT_h = (x A2_h) x^T / 64   [k, q] layout
      u_h  = x @ (W3_h Wout_h)
      gate = sum_h (E_h^T u_h) / (E_h^T 1),  E_h = exp(scoresT_h - c_h)
    (c_h is a per-head shift; it cancels in the ratio and keeps E in a
     narrow range ~[64, 128].)
  * Host precomputes x^T (f16, stacked twice) and u (f16) so the device
    does no input transposes; all DMAs overlap the main stream.
  * Main loop over (key-tile, head-pair) units x 2 q-halves, software
    pipelined: scoresT chunks on PE (fp16, f32 psum), exp on ACT, E (f16)
    streamed back through PE against stationary [u_h | 1] accumulating
    g/rs rows in psum.  Half the odd-kt exp tiles are offloaded from ACT
    to DVE using E ~= (a*s + b)^2 (valid because the score range is tiny,
    |s|/64 < 0.45; the quadratic's rel err ~7e-3 on those tiles only).
  * zt prep and the first q-half's tail (PE transposes of g/rs, gate
    division, out = x * gate, output DMA) are interleaved into the
    stream; psum is partitioned 8KB scores + 4KB g/rs + 4KB shared by
    zt-prep and tail transposes so nothing aliases the hot pools.
"""

import numpy as np

from concourse import bacc, tile
import concourse.mybir as mybir
from concourse.alu_op_type import AluOpType
from concourse.bass_utils import run_bass_kernel_spmd

T = 2048
C = 64
H = 4
F = 256
P = 128
NT = T // P  # 16 key tiles

f32 = mybir.dt.float32
f16 = mybir.dt.float16
AF = mybir.ActivationFunctionType

_NC_CACHE = None


def _offloaded(kt, head):
    # 16 of 64 exp tiles per q-half move from ACT to DVE
    return kt % 2 == 1 and head % 2 == 1


def _build_nc():
    nc = bacc.Bacc("TRN2", target_bir_lowering=False, debug=False)
    x_d = nc.dram_tensor("x", [T, C], f32, kind="ExternalInput").ap()
    xt_d = nc.dram_tensor("xt2", [P, T], f16, kind="ExternalInput").ap()
    zt_d = [
        nc.dram_tensor(f"zt{i}", [P, T], f16, kind="ExternalInput").ap()
        for i in range(2)
    ]
    u_d = nc.dram_tensor("u16", [P, NT, 2, H], f16, kind="ExternalInput").ap()
    id_d = nc.dram_tensor("ident", [P, P], f32, kind="ExternalInput").ap()
    cf_d = nc.dram_tensor("coef", [P, 12], f32, kind="ExternalInput").ap()
    y_d = nc.dram_tensor("y", [T, C], f32, kind="ExternalOutput").ap()

    with tile.TileContext(nc) as tc:
        with tc.tile_pool(name="per", bufs=1) as per:
            xT2 = per.tile([P, T], f16, tag="xT2")
            x_sb = per.tile([P, NT, C], f32, tag="x_sb")
            u16 = per.tile([P, NT, 2, H], f16, tag="u16")
            id_sb = per.tile([P, P], f32, tag="id_sb")
            cf = per.tile([P, 12], f32, tag="cf")
            zt = [
                per.tile([P, T], f16, tag=f"zt{i}", name=f"zt{i}") for i in range(2)
            ]
            z1 = per.tile([1, P], f16, tag="z1")
            z512 = per.tile([1, 512], f16, tag="z512")
            t_sb = per.tile([P, 2, 1024], f32, tag="t_sb")
            gate = per.tile([P, NT], f32, tag="gate")
            y_sb = per.tile([P, NT, C], f32, tag="y_sb")
            warm = per.tile([P, 1], f32, tag="warm")

            # DMA order: the stream's critical path spread over 4 queues
            # (xT chunks 0/1 and a2h feed the first unit); x (tail-only)
            # last.
            nc.sync.dma_start(zt[0][:, 0:512], zt_d[0][:, 0:512])
            nc.gpsimd.dma_start(zt[1][:, 0:512], zt_d[1][:, 0:512])
            nc.scalar.dma_start(xT2[0:C, 0:1024], xt_d[0:C, 0:1024])
            nc.scalar.dma_start(xT2[C:P, 0:1024], xt_d[C:P, 0:1024])
            nc.sync.dma_start(cf[:], cf_d[:])
            nc.sync.dma_start(u16[:], u_d[:])
            # bulk, ordered by first-use deadline (kt4 ~18us, kt8 ~26us,
            # q-pass 1 ~70us, tail ~75us)
            nc.sync.dma_start(zt[0][:, 512:1024], zt_d[0][:, 512:1024])
            nc.gpsimd.dma_start(zt[1][:, 512:1024], zt_d[1][:, 512:1024])
            nc.sync.dma_start(zt[0][:, 1024:2048], zt_d[0][:, 1024:2048])
            nc.gpsimd.dma_start(zt[1][:, 1024:2048], zt_d[1][:, 1024:2048])
            nc.scalar.dma_start(xT2[:, 1024:2048], xt_d[:, 1024:2048])
            nc.gpsimd.dma_start(id_sb[:], id_d[:])
            for g in range(4):
                q = [nc.sync, nc.gpsimd, nc.scalar, nc.scalar][g]
                q.dma_start(
                    x_sb[:, g * 4:(g + 1) * 4, :],
                    x_d[g * 512:(g + 1) * 512, :].rearrange("(j p) c -> p j c", p=P),
                )

            # Load the exp table on ACT early (~2.7us) so it overlaps prep.
            nc.vector.memset(warm[:], 0.0)
            nc.scalar.activation(warm[:], warm[:], AF.Exp, scale=1.0)

            nc.vector.memset(z1[:], 0.0)
            nc.vector.memset(z512[:], 0.0)


            with (
                tc.tile_pool(name="ps_s", bufs=3, space="PSUM") as pss,
                tc.tile_pool(name="ps_grs", bufs=1, space="PSUM") as psg,
                tc.tile_pool(name="e_pool", bufs=8) as ep,
                tc.tile_pool(name="t_pool", bufs=4) as tp,
                tc.tile_pool(name="tailsb", bufs=2) as tsb,
            ):
                pending_i2 = []

                def flush_i2():
                    while pending_i2:
                        e, t16 = pending_i2.pop(0)
                        nc.vector.tensor_mul(e[:], t16[:], t16[:])

                def emit_exp(ps, kt, h):
                    e = ep.tile([P, 1024], f16, tag="e", name="e")
                    if _offloaded(kt, h):
                        # i1 frees the psum slot; the square (i2) is
                        # deferred so i1s keep DVE-queue priority.
                        t16 = tp.tile([P, 1024], f16, tag="t16", name="t16")
                        nc.vector.tensor_scalar(
                            t16[:],
                            ps[:],
                            cf[:, 4 + h:5 + h],
                            cf[:, 8 + h:9 + h],
                            AluOpType.mult,
                            AluOpType.add,
                        )
                        pending_i2.append((e, t16))
                    else:
                        nc.scalar.activation(
                            e[:], ps[:], AF.Exp, bias=cf[:, h:h + 1], scale=1.0 / 64.0
                        )
                    return e

                def emit_scores_exp(qpass, kt, pair):
                    psA = pss.tile([P, 1024], f32, tag="ps_s", name="psA")
                    psB = pss.tile([P, 1024], f32, tag="ps_s", name="psB")
                    for sub in range(2):
                        q0 = qpass * 1024 + sub * 512
                        nc.tensor.matmul(
                            psA[:, sub * 512:(sub + 1) * 512],
                            zt[pair][0:C, kt * P:(kt + 1) * P],
                            xT2[0:C, q0:q0 + 512],
                            start=True,
                            stop=True,
                        )
                        nc.tensor.matmul(
                            psB[:, sub * 512:(sub + 1) * 512],
                            zt[pair][C:P, kt * P:(kt + 1) * P],
                            xT2[C:P, q0:q0 + 512],
                            start=True,
                            stop=True,
                        )
                    eA = emit_exp(psA, kt, 2 * pair)
                    eB = emit_exp(psB, kt, 2 * pair + 1)
                    return eA, eB

                def emit_pass2(grs, kt, pair, eA, eB, last, first=False):
                    flush_i2()
                    hA, hB = 2 * pair, 2 * pair + 1
                    for sub in range(2):
                        nc.tensor.matmul(
                            grs[32 * hA:32 * hA + 2, sub * 512:(sub + 1) * 512],
                            u16[:, kt, :, hA],
                            eA[:, sub * 512:(sub + 1) * 512],
                            start=False,
                            stop=last,
                            skip_group_check=True,
                            tile_position=(0, 32 * hA),
                        )
                        nc.tensor.matmul(
                            grs[32 * hB:32 * hB + 2, sub * 512:(sub + 1) * 512],
                            u16[:, kt, :, hB],
                            eB[:, sub * 512:(sub + 1) * 512],
                            start=False,
                            stop=last,
                            skip_group_check=True,
                            tile_position=(0, 32 * hB),
                        )

                def emit_tail(qt0, n, queues, per_tile_dma):
                    half = qt0 // 8
                    tg = pss.tile([P, n, P], f32, tag="ps_s", name="tg")
                    for j in range(n):
                        lcl = qt0 + j - half * 8
                        nc.tensor.transpose(
                            tg[:, j, :],
                            t_sb[:, half, lcl * P:(lcl + 1) * P],
                            id_sb[:],
                        )
                    tgr = tg[:].rearrange("p f (h j) -> p f h j", h=4)
                    rec = tsb.tile([P, n, H], f32, tag="rec", name="rec", bufs=2)
                    nc.vector.reciprocal(rec[:], tgr[:, :, :, 1])
                    gm = tsb.tile([P, n, H], f32, tag="gm", name="gm", bufs=2)
                    nc.vector.tensor_mul(gm[:], tgr[:, :, :, 0], rec[:])
                    nc.vector.tensor_reduce(
                        gate[:, qt0:qt0 + n],
                        gm[:],
                        axis=mybir.AxisListType.X,
                        op=mybir.AluOpType.add,
                    )
                    for j in range(n):
                        qt = qt0 + j
                        nc.vector.tensor_scalar_mul(
                            y_sb[:, qt, :], x_sb[:, qt, :], gate[:, qt:qt + 1]
                        )
                        if per_tile_dma:
                            queues[j % len(queues)].dma_start(
                                y_d[qt * P:(qt + 1) * P, :], y_sb[:, qt, :]
                            )
                    if not per_tile_dma:
                        queues[0].dma_start(
                            y_d[qt0 * P:(qt0 + n) * P, :].rearrange(
                                "(j p) c -> p j c", p=P
                            ),
                            y_sb[:, qt0:qt0 + n, :],
                        )

                for qpass in range(2):
                    grs = psg.tile([P, 1024], f32, tag="grs", name="grs")

                    def emit_seeds():
                        for c in range(2):
                            nc.tensor.matmul(
                                grs[:, c * 512:(c + 1) * 512],
                                z1[:],
                                z512[:],
                                start=True,
                                stop=False,
                                skip_group_check=True,
                            )

                    if qpass == 0:
                        emit_seeds()
                    kts = [1, 0] + list(range(2, 14)) + [15, 14]
                    units = [(kt, pair) for kt in kts for pair in range(2)]
                    prev = None
                    for idx, unit in enumerate(units):
                        e_tiles = emit_scores_exp(qpass, *unit)
                        if qpass == 1 and idx == 0:
                            emit_seeds()  # deferred past the evac WAR
                        if qpass == 1 and idx == 4:
                            # half-0 tail overlaps q-pass 1
                            emit_tail(0, 4, [nc.sync], False)
                        if qpass == 1 and idx == 8:
                            emit_tail(4, 4, [nc.gpsimd], False)
                        if prev is not None:
                            emit_pass2(grs, *prev[0], *prev[1], last=False)
                        prev = (unit, e_tiles)
                    emit_pass2(grs, *prev[0], *prev[1], last=True)
                    if qpass == 0:
                        nc.vector.tensor_copy(t_sb[:, 0, :], grs[:])
                    else:
                        # fine-grained evac + 2-tile tail groups: shortest
                        # possible serial chain after the last exp
                        for gg in range(4):
                            c0 = gg * 256
                            nc.vector.tensor_copy(
                                t_sb[:, 1, c0:c0 + 256], grs[:, c0:c0 + 256]
                            )
                            emit_tail(8 + 2 * gg, 2,
                                      [nc.sync, nc.scalar], True)

    nc.compile()
    return nc


def _get_nc():
    global _NC_CACHE
    if _NC_CACHE is None:
        _NC_CACHE = _build_nc()
    return _NC_CACHE


def _host_prep(inputs_tran, W1, W2, W3, Wout):
    x64 = inputs_tran.astype(np.float64)
    W1r = W1.astype(np.float64).reshape(C, H, C)
    W2r = W2.astype(np.float64).reshape(C, H, C)
    W3r = W3.astype(np.float64).reshape(C, H, C)
    Wor = Wout.astype(np.float64).reshape(H, C)
    a2 = np.einsum("chd,qhd->chq", W2r, W1r)  # [C, H, Cq]
    wt = np.einsum("chd,hd->ch", W3r, Wor)    # [C, H]

    # Per-head score range (s/64) estimated on a q-subsample; the margins
    # below cover the sampling shortfall many times over (scores are
    # Gaussian-ish with sigma ~0.07 in s/64 units).
    z = np.einsum("btc,chq->bthq", x64, a2)   # [B, T, H, C]
    qsel = np.arange(0, T, 8)
    xs = x64[:, qsel, :]                      # [B, 256, C]
    smax = np.zeros(H)
    smin = np.zeros(H)
    for h in range(H):
        ss = np.einsum("btq,bsq->bts", z[:, :, h, :], xs) / 64.0
        smax[h] = ss.max()
        smin[h] = ss.min()

    ln_peak = np.log(128.0)
    coef = np.zeros((P, 12), dtype=np.float32)
    for h in range(H):
        c_h = smax[h] + 0.05 - ln_peak
        lo = smin[h] - c_h - 0.10
        hi = ln_peak + 0.10
        # fit exp(t) ~= (a t + b)^2 on [lo, hi]: weighted lstsq of a*t+b
        # against exp(t/2) (near-minimax in relative error)
        ts = np.linspace(lo, hi, 2001)
        y = np.exp(ts / 2.0)
        A = np.stack([ts / y, 1.0 / y], axis=1)
        (a_h, b_h), *_ = np.linalg.lstsq(A, np.ones_like(ts), rcond=None)
        coef[:, h] = -c_h                      # ACT exp bias
        coef[:, 4 + h] = a_h / 64.0            # DVE quad scale (raw psum s)
        coef[:, 8 + h] = b_h - a_h * c_h       # DVE quad offset

    # zt[b][fh] = [z_{2fh}^T ; z_{2fh+1}^T]  [128, T] f16 per batch
    ztp = np.empty((x64.shape[0], 2, P, T), dtype=np.float16)
    for fh in range(2):
        ztp[:, fh, 0:C, :] = z[:, :, 2 * fh, :].transpose(0, 2, 1)
        ztp[:, fh, C:P, :] = z[:, :, 2 * fh + 1, :].transpose(0, 2, 1)
    u = np.einsum("btc,ch->bth", x64, wt)      # [B, T, H]
    return ztp, u, coef


def _run(inputs_tran, W1, W2, W3, Wout, trace=False):
    nc = _get_nc()
    ztp, u, coef = _host_prep(inputs_tran, W1, W2, W3, Wout)
    ident = np.eye(P, dtype=np.float32)
    B = inputs_tran.shape[0]
    in_maps = []
    for b in range(B):
        xb = np.ascontiguousarray(inputs_tran[b], dtype=np.float32)
        xt2 = np.concatenate([xb.T, xb.T], axis=0).astype(np.float16)  # [128, T]
        u16 = np.empty((P, NT, 2, H), dtype=np.float16)
        # u16[p, kt, 0, h] = u_h at key kt*128+p
        u16[:, :, 0, :] = u[b].reshape(NT, P, H).transpose(1, 0, 2).astype(np.float16)
        u16[:, :, 1, :] = np.float16(1.0)
        in_maps.append(
            {
                "x": xb,
                "xt2": xt2,
                "zt0": np.ascontiguousarray(ztp[b, 0]),
                "zt1": np.ascontiguousarray(ztp[b, 1]),
                "u16": u16,
                "ident": ident,
                "coef": coef,
            }
        )
    res = run_bass_kernel_spmd(nc, in_maps, list(range(B)), trace=trace)
    out = np.stack([res.results[b]["y"] for b in range(B)], axis=0)
    return out.astype(np.float32), res


def kernel(inputs_tran, W1, W2, W3, Wout):
    out, _ = _run(inputs_tran, W1, W2, W3, Wout, trace=False)
    return out
